# revision 1
# baseline (speedup 1.0000x reference)
"""Trainium2 Bass kernel for nn_GAT_FP (3-layer GAT message passing), 8 cores.

Sharding: nodes split 1250/core (dst-owner). Edges sorted by dst, grouped
into 10 windows of 128 consecutive owned dst rows per core. Per window one
dma_gather pulls all source-node feature rows; dst-side expansion and
segment-sum run as selection-matrix matmuls on the PE. Source feature
tables (fs / zes / fs1) are AllGathered after the dense projections.
Feature masks are folded into the weight matrices on the host; bias rows
are augmented into the contraction dim. Segment softmax skips the
max-subtraction (logits are O(1); exp cannot overflow). The wide dense
path (h, layer-0/1 weights, fs/fd tables) runs in bf16 with fp32 PSUM
accumulation; preprocessing, softmax and epilogues stay fp32.
"""
import sys
sys.path.insert(0, "/opt/trn_rl_repo")
import math
import numpy as np
import ml_dtypes

import concourse.bass as bass
import concourse.tile as tile
from concourse import bacc, mybir
from concourse.bass_utils import run_bass_kernel_spmd
from concourse.masks import make_identity

F32 = mybir.dt.float32
BF16 = mybir.dt.bfloat16
I16 = mybir.dt.int16
I32 = mybir.dt.int32
AF = mybir.ActivationFunctionType
OP = mybir.AluOpType
AX = mybir.AxisListType
NPBF = ml_dtypes.bfloat16

N, E, IN = 10000, 64000, 1247
H, D0, D1, OUT = 4, 256, 8, 6
HD0, HD1 = H * D0, H * D1          # 1024, 32
NC = 8
NPC = N // NC                       # 1250 nodes per core
WPC = (NPC + 127) // 128            # 10 windows per core
KA = IN + 1                         # 1248 augmented contraction dim
K0T = (KA + 127) // 128             # 10 k-tiles layer-0 dense
K1T = HD0 // 128                    # 8 k-tiles layer-1 dense
NEG = -30000.0                      # pad logit bias -> exp == 0

_compiled = {}
last_exec_ns = None
_last_in_maps = None


def _wrows(w):
    return min(128, NPC - w * 128)


def _build_program(Ts, reps=1, comms=True):
    totT = sum(Ts)
    Tmax = max(Ts)
    toff = [sum(Ts[:w]) for w in range(WPC)]
    nc = bacc.Bacc("TRN2", target_bir_lowering=False, debug=False,
                   num_devices=NC)

    feat = nc.dram_tensor("feat", [NPC, IN], F32, kind="ExternalInput")
    wl0a = nc.dram_tensor("wl0a", [KA, HD0], BF16, kind="ExternalInput")
    wr0a = nc.dram_tensor("wr0a", [KA, HD0], BF16, kind="ExternalInput")
    wres0a = nc.dram_tensor("wres0a", [KA, HD0], BF16, kind="ExternalInput")
    w2p = nc.dram_tensor("w2p", [KA, HD1], BF16, kind="ExternalInput")
    wl1 = nc.dram_tensor("wl1", [HD0, HD1], BF16, kind="ExternalInput")
    wr1 = nc.dram_tensor("wr1", [HD0, HD1], BF16, kind="ExternalInput")
    wres1 = nc.dram_tensor("wres1", [HD0, HD1], BF16, kind="ExternalInput")
    wlin = nc.dram_tensor("wlin", [2 * HD1, OUT], F32, kind="ExternalInput")
    a0bc = nc.dram_tensor("a0bc", [128, HD0], F32, kind="ExternalInput")
    a1bc = nc.dram_tensor("a1bc", [128, HD1], F32, kind="ExternalInput")
    a2sbc = nc.dram_tensor("a2sbc", [128, HD1], F32, kind="ExternalInput")
    a2dbc = nc.dram_tensor("a2dbc", [128, HD1], F32, kind="ExternalInput")
    b1bc = nc.dram_tensor("b1bc", [128, HD1], F32, kind="ExternalInput")
    blinbc = nc.dram_tensor("blinbc", [128, OUT], F32, kind="ExternalInput")
    srcidx = nc.dram_tensor("srcidx", [128, 8 * totT], I16, kind="ExternalInput")
    dstloc = nc.dram_tensor("dstloc", [128, totT], F32, kind="ExternalInput")
    vbias = nc.dram_tensor("vbias", [128, totT], F32, kind="ExternalInput")
    out_ext = nc.dram_tensor("out", [NPC, OUT], F32, kind="ExternalOutput")

    import contextlib

    with tile.TileContext(nc) as tc:
        with tc.tile_pool(name="dram", bufs=1, space="DRAM") as dram, \
             tc.tile_pool(name="constp", bufs=1) as constp, \
             tc.tile_pool(name="hold", bufs=1) as hold, \
             tc.tile_pool(name="work", bufs=2) as work, \
             (tc.For_i(0, reps, 1) if reps > 1 else contextlib.nullcontext()):

            fs_c = dram.tile([NPC, HD0], BF16)
            fd_c = dram.tile([NPC, HD0], BF16)
            res_c = dram.tile([NPC, HD0], F32)
            zes_c = dram.tile([NPC, 64], F32)
            fs1p_c = dram.tile([NPC, 64], F32)
            ASP = "Shared" if comms else "Local"
            fs_full = dram.tile([N, HD0], BF16, addr_space=ASP)
            zes_full = dram.tile([N, 64], F32, addr_space=ASP)
            fs1p_full = dram.tile([N, 64], F32, addr_space=ASP)
            cs_bounce = dram.tile([1, IN], F32)
            cs_sum = dram.tile([1, IN], F32, addr_space=ASP)

            ident = constp.tile([128, 128], F32)
            make_identity(nc, ident[:])
            iota_row_i = constp.tile([128, 128], I32)
            nc.gpsimd.iota(iota_row_i[:], pattern=[[1, 128]], channel_multiplier=0)
            iota_row = constp.tile([128, 128], F32)
            nc.vector.tensor_copy(out=iota_row[:], in_=iota_row_i[:])
            iota_col_i = constp.tile([128, 128], I32)
            nc.gpsimd.iota(iota_col_i[:], pattern=[[0, 128]], channel_multiplier=1)
            iota_col = constp.tile([128, 128], F32)
            nc.vector.tensor_copy(out=iota_col[:], in_=iota_col_i[:])
            ones128 = constp.tile([128, 1], F32)
            nc.vector.memset(ones128[:], 1.0)
            ones_row = constp.tile([1, 128], F32)
            nc.vector.memset(ones_row[:], 1.0)

            def load_const(name, dramt, shape, dt=F32):
                t = constp.tile(shape, dt, tag=name, name=name)
                nc.sync.dma_start(out=t[:], in_=dramt[:])
                return t
            a0b = load_const("a0b", a0bc, [128, HD0])
            a1b = load_const("a1b", a1bc, [128, HD1])
            a2sb = load_const("a2sb", a2sbc, [128, HD1])
            a2db = load_const("a2db", a2dbc, [128, HD1])
            b1b = load_const("b1b", b1bc, [128, HD1])
            blinb = load_const("blinb", blinbc, [128, OUT])
            dlocs = load_const("dlocs", dstloc, [128, totT])
            vbs = load_const("vbs", vbias, [128, totT])
            sidx = load_const("sidx", srcidx, [128, 8 * totT], I16)
            wlsb = load_const("wlsb", wlin, [2 * HD1, OUT])

            catT = hold.tile([64, NPC], F32)
            ed_t = [hold.tile([128, H], BF16, tag=f"ed{m}", name=f"ed{m}")
                    for m in range(WPC)]
            fd1_t = [hold.tile([128, HD1], F32, tag=f"fd1_{m}", name=f"fd1_{m}")
                     for m in range(WPC)]
            res1_t = [hold.tile([128, HD1], F32, tag=f"res1_{m}",
                                name=f"res1_{m}") for m in range(WPC)]

            ncol = [(j * 512, min(512, IN - j * 512))
                    for j in range((IN + 511) // 512)]

            # ============ Phases P, D0, z (hT alive) ============
            with tc.tile_pool(name="hpool", bufs=1) as hpool:
                hT = hpool.tile([128, K0T * NPC], BF16)

                with tc.tile_pool(name="pp", bufs=1) as pp, \
                     tc.tile_pool(name="psP", bufs=1, space="PSUM") as psP:
                    # pass 1: column sums (streamed feature tiles)
                    cs_sb = pp.tile([1, IN], F32, tag="cs_sb")
                    cpss = [psP.tile([1, 512], F32, tag=f"cs{j}", name=f"cs{j}",
                                     space="PSUM") for j in range(len(ncol))]
                    for m in range(WPC):
                        pr = _wrows(m)
                        ft = pp.tile([128, IN], F32, tag="fstream", name="ft",
                                     bufs=3)
                        nc.sync.dma_start(out=ft[:pr, :],
                                          in_=feat[m * 128:m * 128 + pr, :])
                        for j, (c0, cw) in enumerate(ncol):
                            nc.tensor.matmul(out=cpss[j][:1, :cw],
                                             lhsT=ones128[:pr, :],
                                             rhs=ft[:pr, c0:c0 + cw],
                                             start=(m == 0), stop=(m == WPC - 1))
                    for j, (c0, cw) in enumerate(ncol):
                        nc.scalar.copy(out=cs_sb[:, c0:c0 + cw],
                                       in_=cpss[j][:1, :cw])
                    nc.gpsimd.dma_start(out=cs_bounce[:], in_=cs_sb[:])
                    if comms:
                        nc.gpsimd.collective_compute(
                            "AllReduce", OP.add,
                            replica_groups=[list(range(NC))],
                            ins=[cs_bounce[:]], outs=[cs_sum[:]])
                    else:
                        nc.gpsimd.dma_start(out=cs_sum[:], in_=cs_bounce[:])
                    meanh = pp.tile([1, IN], F32, tag="meanh")
                    nc.sync.dma_start(out=meanh[:], in_=cs_sum[:])
                    nc.scalar.mul(out=meanh[:], in_=meanh[:], mul=0.5 / N)
                    meanb = pp.tile([128, IN], F32, tag="meanb")
                    for j, (c0, cw) in enumerate(ncol):
                        bps = psP.tile([128, 512], F32, tag="bps", name="bps",
                                       space="PSUM")
                        nc.tensor.matmul(out=bps[:, :cw], lhsT=ones_row[:, :],
                                         rhs=meanh[:, c0:c0 + cw],
                                         start=True, stop=True)
                        nc.scalar.copy(out=meanb[:, c0:c0 + cw], in_=bps[:, :cw])

                    # pass 2: impute + L1-normalize + transpose into hT (bf16)
                    for m in range(WPC):
                        pr = _wrows(m)
                        ft = pp.tile([128, KA], F32, tag="fstream2", name="ft",
                                     bufs=3)
                        nc.sync.dma_start(out=ft[:pr, 0:IN],
                                          in_=feat[m * 128:m * 128 + pr, :])
                        nc.vector.memset(ft[:, IN:KA], 1.0)
                        msk = pp.tile([128, IN], F32, tag="msk", name="msk",
                                      bufs=2)
                        nc.vector.tensor_scalar(out=msk[:pr, :],
                                                in0=ft[:pr, 0:IN],
                                                scalar1=0.0, scalar2=None,
                                                op0=OP.is_equal)
                        nc.vector.tensor_tensor(out=msk[:pr, :], in0=msk[:pr, :],
                                                in1=meanb[:pr, :], op=OP.mult)
                        nc.vector.tensor_tensor(out=ft[:pr, 0:IN],
                                                in0=ft[:pr, 0:IN],
                                                in1=msk[:pr, :], op=OP.add)
                        rs = work.tile([128, 1], F32, tag="rs")
                        nc.vector.tensor_reduce(out=rs[:pr, :],
                                                in_=ft[:pr, 0:IN],
                                                axis=AX.X, op=OP.add,
                                                apply_absolute_value=True)
                        nc.vector.tensor_scalar(out=rs[:pr, :], in0=rs[:pr, :],
                                                scalar1=1e-12, scalar2=None,
                                                op0=OP.max)
                        rinv = work.tile([128, 1], F32, tag="rinv")
                        nc.vector.reciprocal(out=rinv[:pr, :], in_=rs[:pr, :])
                        nc.vector.tensor_scalar(out=ft[:pr, 0:IN],
                                                in0=ft[:pr, 0:IN],
                                                scalar1=rinv[:pr, 0:1],
                                                scalar2=None, op0=OP.mult)
                        for k in range(K0T):
                            kw = min(128, KA - k * 128)
                            tps = psP.tile([128, 128], F32, tag="tps", name="tps",
                                           space="PSUM", bufs=2)
                            nc.tensor.transpose(out=tps[:kw, :pr],
                                                in_=ft[:pr, k * 128:k * 128 + kw],
                                                identity=ident[:pr, :pr])
                            nc.scalar.copy(
                                out=hT[:kw, k * NPC + m * 128:
                                       k * NPC + m * 128 + pr],
                                in_=tps[:kw, :pr])

                # ---------- D0 dense (bf16 x bf16 -> f32 psum) ----------
                with tc.tile_pool(name="dpool", bufs=1) as dpool, \
                     tc.tile_pool(name="psD", bufs=2, space="PSUM") as psD:

                    def dense0(wdram, dest, odt):
                        wkt = [dpool.tile([128, HD0], BF16, tag=f"wk{k}",
                                          name=f"wk{k}", bufs=2)
                               for k in range(K0T)]
                        for k in range(K0T):
                            kw = min(128, KA - k * 128)
                            nc.sync.dma_start(out=wkt[k][:kw, :],
                                              in_=wdram[k * 128:k * 128 + kw, :])
                        for m in range(WPC):
                            pr = _wrows(m)
                            osb = dpool.tile([128, HD0], odt, tag=f"d0o{odt}",
                                             name="d0o", bufs=2)
                            for j in range(2):
                                ops = psD.tile([128, 512], F32, tag="d0ps",
                                               name="d0ps", space="PSUM")
                                for k in range(K0T):
                                    kw = min(128, KA - k * 128)
                                    nc.tensor.matmul(
                                        out=ops[:pr, :],
                                        lhsT=hT[:kw, k * NPC + m * 128:
                                                k * NPC + m * 128 + pr],
                                        rhs=wkt[k][:kw, j * 512:(j + 1) * 512],
                                        start=(k == 0), stop=(k == K0T - 1))
                                nc.scalar.copy(
                                    out=osb[:pr, j * 512:(j + 1) * 512],
                                    in_=ops[:pr, :])
                            nc.sync.dma_start(out=dest[m * 128:m * 128 + pr, :],
                                              in_=osb[:pr, :])

                    dense0(wl0a, fs_c, BF16)
                    if comms:
                        nc.gpsimd.collective_compute(
                            "AllGather", OP.bypass,
                            replica_groups=[list(range(NC))],
                            ins=[fs_c[:]], outs=[fs_full[:]])
                    else:
                        for r in range(NC):
                            nc.sync.dma_start(
                                out=fs_full[r * NPC:(r + 1) * NPC, :],
                                in_=fs_c[:, :])
                    dense0(wr0a, fd_c, BF16)
                    dense0(wres0a, res_c, F32)

                    w2sb = dpool.tile([128, K0T * HD1], BF16, tag="w2sb")
                    for k in range(K0T):
                        kw = min(128, KA - k * 128)
                        nc.sync.dma_start(out=w2sb[:kw, k * HD1:(k + 1) * HD1],
                                          in_=w2p[k * 128:k * 128 + kw, :])
                    for m in range(WPC):
                        pr = _wrows(m)
                        zps = psD.tile([128, HD1], F32, tag="zps", name="zps",
                                       space="PSUM")
                        for k in range(K0T):
                            kw = min(128, KA - k * 128)
                            nc.tensor.matmul(
                                out=zps[:pr, :],
                                lhsT=hT[:kw, k * NPC + m * 128:
                                        k * NPC + m * 128 + pr],
                                rhs=w2sb[:kw, k * HD1:(k + 1) * HD1],
                                start=(k == 0), stop=(k == K0T - 1))
                        zsb = dpool.tile([128, 64], F32, tag="zsb", name="zsb",
                                         bufs=2)
                        nc.vector.memset(zsb[:], 0.0)
                        nc.scalar.copy(out=zsb[:pr, 0:HD1], in_=zps[:pr, :])
                        tmp = dpool.tile([128, HD1], F32, tag="ztmp", name="ztmp",
                                         bufs=2)
                        nc.vector.tensor_tensor(out=tmp[:pr, :],
                                                in0=zsb[:pr, 0:HD1],
                                                in1=a2sb[:pr, :], op=OP.mult)
                        nc.vector.tensor_reduce(
                            out=zsb[:pr, 32:36],
                            in_=tmp[:pr, :].rearrange("p (h d) -> p h d", h=H),
                            axis=AX.X, op=OP.add)
                        edf = dpool.tile([128, H], F32, tag="edf", name="edf",
                                         bufs=2)
                        nc.vector.tensor_tensor(out=tmp[:pr, :],
                                                in0=zsb[:pr, 0:HD1],
                                                in1=a2db[:pr, :], op=OP.mult)
                        nc.vector.tensor_reduce(
                            out=edf[:pr, :],
                            in_=tmp[:pr, :].rearrange("p (h d) -> p h d", h=H),
                            axis=AX.X, op=OP.add)
                        nc.vector.tensor_copy(out=ed_t[m][:pr, :],
                                              in_=edf[:pr, :])
                        nc.sync.dma_start(out=zes_c[m * 128:m * 128 + pr, :],
                                          in_=zsb[:pr, :])
                    if comms:
                        nc.gpsimd.collective_compute(
                            "AllGather", OP.bypass,
                            replica_groups=[list(range(NC))],
                            ins=[zes_c[:]], outs=[zes_full[:]])
                    else:
                        for r in range(NC):
                            nc.sync.dma_start(
                                out=zes_full[r * NPC:(r + 1) * NPC, :],
                                in_=zes_c[:, :])

            # ============ shared sel-matrix builder ============
            def build_sel(t, dl, pspool, e0mode):
                sel = work.tile([128, 128], F32, tag=f"sel{t}", name=f"sel{t}")
                nc.vector.tensor_tensor(out=sel[:],
                                        in0=dl.to_broadcast([128, 128]),
                                        in1=iota_row[:], op=OP.is_equal)
                dps = pspool.tile([128, 128], F32, tag="dps", name="dps",
                                  space="PSUM")
                nc.tensor.transpose(out=dps[:], in_=dl.to_broadcast([128, 128]),
                                    identity=ident[:])
                dlT = work.tile([128, 128], F32, tag="dlT")
                nc.scalar.copy(out=dlT[:], in_=dps[:])
                if e0mode:
                    selT16 = work.tile([128, 128], BF16, tag="selT16",
                                       name="selT16")
                    nc.vector.tensor_tensor(out=selT16[:], in0=iota_col[:],
                                            in1=dlT[:], op=OP.is_equal)
                    sel16 = work.tile([128, 128], BF16, tag=f"sel16_{t}",
                                      name=f"sel16_{t}")
                    nc.vector.tensor_copy(out=sel16[:], in_=sel[:])
                    return sel, sel16, selT16
                selT = work.tile([128, 128], F32, tag="selT", name="selT")
                nc.vector.tensor_tensor(out=selT[:], in0=iota_col[:], in1=dlT[:],
                                        op=OP.is_equal)
                return sel, selT

            # ============ Phase E0 + inner (h1T alive through D1) ============
            with tc.tile_pool(name="h1pool", bufs=1) as h1pool:
                h1T = h1pool.tile([128, K1T * NPC], BF16)
                with tc.tile_pool(name="psE0", bufs=1, space="PSUM") as psE0:
                    for w in range(WPC):
                        T = Ts[w]
                        nloc = _wrows(w)
                        co = toff[w]
                        fsg = work.tile([128, Tmax * HD0], BF16, tag="fsg")
                        nc.gpsimd.dma_gather(
                            out_ap=fsg[:].rearrange("p (t e) -> p t e",
                                                    t=Tmax)[:, :T, :],
                            in_ap=fs_full[:],
                            idxs_ap=sidx[:, 8 * co:8 * (co + T)],
                            num_idxs=T * 128, num_idxs_reg=T * 128,
                            elem_size=HD0)
                        zesg = work.tile([128, Tmax * 64], F32, tag="zesg")
                        nc.gpsimd.dma_gather(
                            out_ap=zesg[:].rearrange("p (t e) -> p t e",
                                                     t=Tmax)[:, :T, :],
                            in_ap=zes_full[:],
                            idxs_ap=sidx[:, 8 * co:8 * (co + T)],
                            num_idxs=T * 128, num_idxs_reg=T * 128,
                            elem_size=64)
                        fdw = work.tile([128, HD0], BF16, tag="fdw")
                        nc.sync.dma_start(out=fdw[:nloc, :],
                                          in_=fd_c[w * 128:w * 128 + nloc, :])
                        resw = work.tile([128, HD0], F32, tag="resw")
                        nc.sync.dma_start(out=resw[:nloc, :],
                                          in_=res_c[w * 128:w * 128 + nloc, :])

                        sels, sel16s, els, el2s = [], [], [], []
                        dn_ps = psE0.tile([128, H], F32, tag="dn", name="dn",
                                          space="PSUM")
                        dn2_ps = psE0.tile([128, H], F32, tag="dn2", name="dn2",
                                           space="PSUM")
                        for t in range(T):
                            dl = dlocs[:, co + t:co + t + 1]
                            vb = vbs[:, co + t:co + t + 1]
                            sel, sel16, selT16 = build_sel(t, dl, psE0, True)
                            sels.append(sel)
                            sel16s.append(sel16)
                            fdx = psE0.tile([128, HD0], F32, tag="mm1024",
                                            name="fdx", space="PSUM")
                            for j in range(2):
                                nc.tensor.matmul(
                                    out=fdx[:, j * 512:(j + 1) * 512],
                                    lhsT=selT16[:nloc, :],
                                    rhs=fdw[:nloc, j * 512:(j + 1) * 512],
                                    start=True, stop=True)
                            tt = work.tile([128, HD0], F32, tag="tt")
                            nc.vector.tensor_tensor(
                                out=tt[:], in0=fsg[:, t * HD0:(t + 1) * HD0],
                                in1=fdx[:], op=OP.add)
                            nc.scalar.activation(out=tt[:], in_=tt[:],
                                                 func=AF.Prelu, alpha=0.2)
                            nc.vector.tensor_tensor(out=tt[:], in0=tt[:],
                                                    in1=a0b[:], op=OP.mult)
                            lg = work.tile([128, H], F32, tag="lg")
                            nc.vector.tensor_reduce(
                                out=lg[:],
                                in_=tt[:].rearrange("p (h d) -> p h d", h=H),
                                axis=AX.X, op=OP.add)
                            el = work.tile([128, H], F32, tag=f"el{t}",
                                           name=f"el{t}")
                            nc.scalar.activation(out=el[:], in_=lg[:],
                                                 func=AF.Exp, bias=vb)
                            els.append(el)
                            nc.tensor.matmul(out=dn_ps[:], lhsT=sel[:, :],
                                             rhs=el[:], start=(t == 0),
                                             stop=(t == T - 1))
                            edx = psE0.tile([128, HD1], F32, tag="m32",
                                            name="edx", space="PSUM")
                            nc.tensor.matmul(out=edx[:, 0:H],
                                             lhsT=selT16[:nloc, :],
                                             rhs=ed_t[w][:nloc, :],
                                             start=True, stop=True)
                            lg2 = work.tile([128, H], F32, tag="lg2")
                            nc.vector.tensor_tensor(
                                out=lg2[:],
                                in0=zesg[:, t * 64 + 32:t * 64 + 36],
                                in1=edx[:, 0:H], op=OP.add)
                            nc.scalar.activation(out=lg2[:], in_=lg2[:],
                                                 func=AF.Prelu, alpha=0.2)
                            el2 = work.tile([128, H], F32, tag=f"el2_{t}",
                                            name=f"el2_{t}")
                            nc.scalar.activation(out=el2[:], in_=lg2[:],
                                                 func=AF.Exp, bias=vb)
                            el2s.append(el2)
                            nc.tensor.matmul(out=dn2_ps[:], lhsT=sel[:, :],
                                             rhs=el2[:], start=(t == 0),
                                             stop=(t == T - 1))

                        idn = work.tile([128, H], F32, tag="idn")
                        nc.vector.tensor_scalar(out=idn[:], in0=dn_ps[:],
                                                scalar1=1e-9, scalar2=None,
                                                op0=OP.max)
                        nc.vector.reciprocal(out=idn[:], in_=idn[:])
                        idn2 = work.tile([128, H], F32, tag="idn2")
                        nc.vector.tensor_scalar(out=idn2[:], in0=dn2_ps[:],
                                                scalar1=1e-9, scalar2=None,
                                                op0=OP.max)
                        nc.vector.reciprocal(out=idn2[:], in_=idn2[:])

                        o_ps = psE0.tile([128, HD0], F32, tag="mm1024",
                                         name="o_ps", space="PSUM")
                        oz_ps = psE0.tile([128, HD1], F32, tag="m32",
                                          name="oz_ps", space="PSUM")
                        for t in range(T):
                            fv = fsg[:, t * HD0:(t + 1) * HD0].rearrange(
                                "p (h d) -> p h d", h=H)
                            nc.vector.tensor_tensor(
                                out=fv, in0=fv,
                                in1=els[t][:].to_broadcast([128, H, D0]),
                                op=OP.mult)
                            zv = zesg[:, t * 64:t * 64 + HD1].rearrange(
                                "p (h d) -> p h d", h=H)
                            nc.vector.tensor_tensor(
                                out=zv, in0=zv,
                                in1=el2s[t][:].to_broadcast([128, H, D1]),
                                op=OP.mult)
                            for j in range(2):
                                nc.tensor.matmul(
                                    out=o_ps[:, j * 512:(j + 1) * 512],
                                    lhsT=sel16s[t][:, :],
                                    rhs=fsg[:, t * HD0 + j * 512:
                                            t * HD0 + (j + 1) * 512],
                                    start=(t == 0), stop=(t == T - 1))
                            nc.tensor.matmul(out=oz_ps[:], lhsT=sels[t][:, :],
                                             rhs=zesg[:, t * 64:t * 64 + HD1],
                                             start=(t == 0), stop=(t == T - 1))

                        ho = work.tile([128, HD0], F32, tag="ho")
                        nc.vector.tensor_tensor(
                            out=ho[:nloc, :].rearrange("p (h d) -> p h d", h=H),
                            in0=o_ps[:nloc, :].rearrange("p (h d) -> p h d", h=H),
                            in1=idn[:nloc, :].to_broadcast([nloc, H, D0]),
                            op=OP.mult)
                        nc.vector.tensor_tensor(out=ho[:nloc, :],
                                                in0=ho[:nloc, :],
                                                in1=resw[:nloc, :], op=OP.add)
                        nc.scalar.activation(out=ho[:nloc, :], in_=ho[:nloc, :],
                                             func=AF.Relu)
                        for k in range(K1T):
                            tps = psE0.tile([128, 128], F32, tag="tpsE",
                                            name="tpsE", space="PSUM")
                            nc.tensor.transpose(
                                out=tps[:, :nloc],
                                in_=ho[:nloc, k * 128:(k + 1) * 128],
                                identity=ident[:nloc, :nloc])
                            nc.scalar.copy(
                                out=h1T[:, k * NPC + w * 128:
                                        k * NPC + w * 128 + nloc],
                                in_=tps[:, :nloc])
                        ozs = work.tile([128, HD1], F32, tag="ozs")
                        nc.vector.tensor_tensor(
                            out=ozs[:nloc, :].rearrange("p (h d) -> p h d", h=H),
                            in0=oz_ps[:nloc, :].rearrange("p (h d) -> p h d", h=H),
                            in1=idn2[:nloc, :].to_broadcast([nloc, H, D1]),
                            op=OP.mult)
                        zt = psE0.tile([128, 128], F32, tag="tpsE", name="zt",
                                       space="PSUM")
                        nc.tensor.transpose(out=zt[:HD1, :nloc],
                                            in_=ozs[:nloc, :],
                                            identity=ident[:nloc, :nloc])
                        nc.scalar.copy(out=catT[0:HD1, w * 128:w * 128 + nloc],
                                       in_=zt[:HD1, :nloc])

                # ---------- D1 dense ----------
                with tc.tile_pool(name="d1pool", bufs=1) as d1pool, \
                     tc.tile_pool(name="psD1", bufs=2, space="PSUM") as psD1:
                    wk1 = {}
                    for nm, wd in (("l", wl1), ("r", wr1), ("res", wres1)):
                        t = d1pool.tile([128, K1T * HD1], BF16, tag=f"wk1{nm}",
                                        name=f"wk1{nm}")
                        for k in range(K1T):
                            nc.sync.dma_start(out=t[:, k * HD1:(k + 1) * HD1],
                                              in_=wd[k * 128:(k + 1) * 128, :])
                        wk1[nm] = t
                    for m in range(WPC):
                        pr = _wrows(m)
                        outs = {}
                        for nm in ("l", "r", "res"):
                            p1 = psD1.tile([128, HD1], F32, tag=f"d1{nm}",
                                           name=f"d1{nm}", space="PSUM")
                            for k in range(K1T):
                                nc.tensor.matmul(
                                    out=p1[:pr, :],
                                    lhsT=h1T[:, k * NPC + m * 128:
                                             k * NPC + m * 128 + pr],
                                    rhs=wk1[nm][:, k * HD1:(k + 1) * HD1],
                                    start=(k == 0), stop=(k == K1T - 1))
                            outs[nm] = p1
                        f1 = d1pool.tile([128, 64], F32, tag="f1", name="f1",
                                         bufs=2)
                        nc.vector.memset(f1[:], 0.0)
                        nc.scalar.copy(out=f1[:pr, 0:HD1], in_=outs["l"][:pr, :])
                        nc.sync.dma_start(out=fs1p_c[m * 128:m * 128 + pr, :],
                                          in_=f1[:pr, :])
                        nc.scalar.copy(out=fd1_t[m][:pr, :],
                                       in_=outs["r"][:pr, :])
                        nc.scalar.copy(out=res1_t[m][:pr, :],
                                       in_=outs["res"][:pr, :])
                    if comms:
                        nc.gpsimd.collective_compute(
                            "AllGather", OP.bypass,
                            replica_groups=[list(range(NC))],
                            ins=[fs1p_c[:]], outs=[fs1p_full[:]])
                    else:
                        for r in range(NC):
                            nc.sync.dma_start(
                                out=fs1p_full[r * NPC:(r + 1) * NPC, :],
                                in_=fs1p_c[:, :])

            # ============ Phase E1 (all f32) ============
            with tc.tile_pool(name="psE1", bufs=1, space="PSUM") as psE1:
                for w in range(WPC):
                    T = Ts[w]
                    nloc = _wrows(w)
                    co = toff[w]
                    f1g = work.tile([128, Tmax * 64], F32, tag="f1g")
                    nc.gpsimd.dma_gather(
                        out_ap=f1g[:].rearrange("p (t e) -> p t e",
                                                t=Tmax)[:, :T, :],
                        in_ap=fs1p_full[:],
                        idxs_ap=sidx[:, 8 * co:8 * (co + T)],
                        num_idxs=T * 128, num_idxs_reg=T * 128, elem_size=64)
                    sels, els = [], []
                    dn_ps = psE1.tile([128, H], F32, tag="dn3", name="dn3",
                                      space="PSUM")
                    for t in range(T):
                        dl = dlocs[:, co + t:co + t + 1]
                        vb = vbs[:, co + t:co + t + 1]
                        sel, selT = build_sel(t, dl, psE1, False)
                        sels.append(sel)
                        fdx = psE1.tile([128, HD1], F32, tag="m32b", name="fdx1",
                                        space="PSUM")
                        nc.tensor.matmul(out=fdx[:], lhsT=selT[:nloc, :],
                                         rhs=fd1_t[w][:nloc, :], start=True,
                                         stop=True)
                        tt = work.tile([128, HD1], F32, tag="tt1")
                        nc.vector.tensor_tensor(out=tt[:],
                                                in0=f1g[:, t * 64:t * 64 + HD1],
                                                in1=fdx[:], op=OP.add)
                        nc.scalar.activation(out=tt[:], in_=tt[:], func=AF.Prelu,
                                             alpha=0.2)
                        nc.vector.tensor_tensor(out=tt[:], in0=tt[:], in1=a1b[:],
                                                op=OP.mult)
                        lg = work.tile([128, H], F32, tag="lg3")
                        nc.vector.tensor_reduce(
                            out=lg[:],
                            in_=tt[:].rearrange("p (h d) -> p h d", h=H),
                            axis=AX.X, op=OP.add)
                        el = work.tile([128, H], F32, tag=f"el3_{t}",
                                       name=f"el3_{t}")
                        nc.scalar.activation(out=el[:], in_=lg[:], func=AF.Exp,
                                             bias=vb)
                        els.append(el)
                        nc.tensor.matmul(out=dn_ps[:], lhsT=sel[:, :], rhs=el[:],
                                         start=(t == 0), stop=(t == T - 1))
                    idn = work.tile([128, H], F32, tag="idn3")
                    nc.vector.tensor_scalar(out=idn[:], in0=dn_ps[:],
                                            scalar1=1e-9, scalar2=None,
                                            op0=OP.max)
                    nc.vector.reciprocal(out=idn[:], in_=idn[:])
                    o_ps = psE1.tile([128, HD1], F32, tag="m32b", name="o_ps1",
                                     space="PSUM")
                    for t in range(T):
                        f1v = f1g[:, t * 64:t * 64 + HD1].rearrange(
                            "p (h d) -> p h d", h=H)
                        nc.vector.tensor_tensor(
                            out=f1v, in0=f1v,
                            in1=els[t][:].to_broadcast([128, H, D1]),
                            op=OP.mult)
                        nc.tensor.matmul(out=o_ps[:], lhsT=sels[t][:, :],
                                         rhs=f1g[:, t * 64:t * 64 + HD1],
                                         start=(t == 0), stop=(t == T - 1))
                    oo = work.tile([128, HD1], F32, tag="oo")
                    nc.vector.tensor_tensor(
                        out=oo[:nloc, :].rearrange("p (h d) -> p h d", h=H),
                        in0=o_ps[:nloc, :].rearrange("p (h d) -> p h d", h=H),
                        in1=idn[:nloc, :].to_broadcast([nloc, H, D1]),
                        op=OP.mult)
                    nc.vector.tensor_tensor(out=oo[:nloc, :], in0=oo[:nloc, :],
                                            in1=res1_t[w][:nloc, :], op=OP.add)
                    nc.vector.tensor_tensor(out=oo[:nloc, :], in0=oo[:nloc, :],
                                            in1=b1b[:nloc, :], op=OP.add)
                    nc.scalar.activation(out=oo[:nloc, :], in_=oo[:nloc, :],
                                         func=AF.Relu)
                    tp = psE1.tile([128, 128], F32, tag="tp1", name="tp1",
                                   space="PSUM")
                    nc.tensor.transpose(out=tp[:HD1, :nloc], in_=oo[:nloc, :],
                                        identity=ident[:nloc, :nloc])
                    nc.scalar.copy(out=catT[HD1:2 * HD1, w * 128:w * 128 + nloc],
                                   in_=tp[:HD1, :nloc])

            # ============ Phase F ============
            with tc.tile_pool(name="psF", bufs=2, space="PSUM") as psF:
                for m in range(WPC):
                    pr = _wrows(m)
                    fp = psF.tile([128, OUT], F32, tag="fin", name="fin",
                                  space="PSUM")
                    nc.tensor.matmul(out=fp[:pr, :],
                                     lhsT=catT[:, m * 128:m * 128 + pr],
                                     rhs=wlsb[:], start=True, stop=True)
                    osb = work.tile([128, OUT], F32, tag="osb")
                    nc.vector.tensor_tensor(out=osb[:pr, :], in0=fp[:pr, :],
                                            in1=blinb[:pr, :], op=OP.add)
                    nc.sync.dma_start(out=out_ext[m * 128:m * 128 + pr, :],
                                      in_=osb[:pr, :])

    nc.compile()
    return nc


def _prep_edges(src, dst):
    order = np.argsort(dst, kind="stable")
    ss = src[order].astype(np.int64)
    ds = dst[order].astype(np.int64)
    cnt = np.zeros((NC, WPC), np.int64)
    bounds = {}
    for c in range(NC):
        for w in range(WPC):
            lo = c * NPC + w * 128
            hi = min(c * NPC + (w + 1) * 128, (c + 1) * NPC)
            e0 = np.searchsorted(ds, lo, side="left")
            e1 = np.searchsorted(ds, hi, side="left")
            cnt[c, w] = e1 - e0
            bounds[(c, w)] = (e0, e1)
    nws = [int(cnt[:, w].max()) for w in range(WPC)]
    Ts = [max(1, math.ceil(nv / 128)) for nv in nws]
    totT = sum(Ts)
    per_core = []
    for c in range(NC):
        sidx = np.zeros((128, 8 * totT), np.int16)
        dloc = np.zeros((128, totT), np.float32)
        vb = np.full((128, totT), NEG, np.float32)
        co = 0
        for w in range(WPC):
            T = Ts[w]
            e0, e1 = bounds[(c, w)]
            k = e1 - e0
            slots = T * 128
            s = np.zeros(slots, np.int16)
            d = np.zeros(slots, np.float32)
            v = np.full(slots, NEG, np.float32)
            s[:k] = ss[e0:e1]
            d[:k] = (ds[e0:e1] - (c * NPC + w * 128)).astype(np.float32)
            v[:k] = 0.0
            cols = s.reshape(8 * T, 16).T
            sidx[:, 8 * co:8 * (co + T)] = np.tile(cols, (8, 1))
            dloc[:, co:co + T] = d.reshape(T, 128).T
            vb[:, co:co + T] = v.reshape(T, 128).T
            co += T
        per_core.append((sidx, dloc, vb))
    return Ts, per_core


def kernel(features, src, dst, textMask, audioMask, videoMask, W2, a2,
           Wl0, Wr0, a0, Wres0, b0, Wl1, Wr1, a1, Wres1, b1, Wlin, blin):
    features = np.asarray(features, np.float32)
    src = np.asarray(src, np.int32)
    dst = np.asarray(dst, np.int32)

    Ts, per_core = _prep_edges(src, dst)
    key = tuple(Ts)
    if key not in _compiled:
        _compiled.clear()
        _compiled[key] = _build_program(Ts)
    nc = _compiled[key]

    maskSum = (np.asarray(textMask) + np.asarray(audioMask)
               + np.asarray(videoMask)).astype(np.float32)

    def aug(Wm, brow=None):
        o = np.zeros((KA, Wm.shape[1]), np.float32)
        o[:IN] = Wm * maskSum[:, None]
        if brow is not None:
            o[IN] = brow
        return o.astype(NPBF)

    w2flat = np.asarray(W2, np.float32).transpose(1, 0, 2).reshape(IN, HD1)
    shared = {
        "wl0a": aug(np.asarray(Wl0, np.float32)),
        "wr0a": aug(np.asarray(Wr0, np.float32)),
        "wres0a": aug(np.asarray(Wres0, np.float32),
                      np.asarray(b0, np.float32)),
        "w2p": aug(w2flat),
        "wl1": np.asarray(Wl1, np.float32).astype(NPBF),
        "wr1": np.asarray(Wr1, np.float32).astype(NPBF),
        "wres1": np.asarray(Wres1, np.float32).astype(NPBF),
        "wlin": np.asarray(Wlin, np.float32),
        "a0bc": np.tile(np.asarray(a0, np.float32).reshape(1, HD0), (128, 1)),
        "a1bc": np.tile(np.asarray(a1, np.float32).reshape(1, HD1), (128, 1)),
        "a2sbc": np.tile(np.asarray(a2, np.float32)[:, :D1].reshape(1, HD1),
                         (128, 1)),
        "a2dbc": np.tile(np.asarray(a2, np.float32)[:, D1:].reshape(1, HD1),
                         (128, 1)),
        "b1bc": np.tile(np.asarray(b1, np.float32).reshape(1, HD1), (128, 1)),
        "blinbc": np.tile(np.asarray(blin, np.float32).reshape(1, OUT),
                          (128, 1)),
    }
    in_maps = []
    for c in range(NC):
        sidx, dloc, vb = per_core[c]
        m = dict(shared)
        m["feat"] = np.ascontiguousarray(features[c * NPC:(c + 1) * NPC])
        m["srcidx"] = sidx
        m["dstloc"] = dloc
        m["vbias"] = vb
        in_maps.append(m)

    global _last_in_maps
    _last_in_maps = in_maps
    res = run_bass_kernel_spmd(nc, in_maps, list(range(NC)))
    global last_exec_ns
    last_exec_ns = getattr(res, "exec_time_ns", None)
    return np.concatenate(
        [np.asarray(res.results[c]["out"]) for c in range(NC)], axis=0)



# revision 23
# speedup vs baseline: 1.0942x; 1.0942x over previous
"""Trainium2 Bass kernel for nn_GAT_FP (3-layer GAT message passing), 8 cores.

Sharding: nodes split 1250/core (dst-owner), edges sorted by dst into 10
windows of 128 dst rows, edge tiles of 128. Per edge tile the GATv2 sum
fs[src]+fd[dst] is built on the PE (selection-matrix expansion plus an
identity matmul over the gathered source rows accumulating into PSUM);
the a0-weighted leaky-relu logit reduction uses activation/tensor_scalar
accum_out over sign-split column ranges (|a0| folded into the layer-0
weights host-side, columns permuted positives-first per head, compensated
in the layer-1 weight rows); the linear part of the leaky-relu comes from
per-node La/Lb columns appended to the fused layer-0 dense. Selection
matrices are precomputed on the host and DMA'd (shared by both GAT
layers). The source-side tables (fs, La, es, z) live in one 1152-column
bf16 row gathered once per window from a single AllGathered buffer. All
transposes ride the DMA crossbar (bf16). Wide math is bf16 with fp32 PSUM.
"""
import sys
sys.path.insert(0, "/opt/trn_rl_repo")
import math
import numpy as np
import ml_dtypes

import concourse.bass as bass
import concourse.tile as tile
from concourse import bacc, mybir
from concourse.bass_utils import run_bass_kernel_spmd
from concourse.masks import make_identity

F32 = mybir.dt.float32
BF16 = mybir.dt.bfloat16
I16 = mybir.dt.int16
AF = mybir.ActivationFunctionType
OP = mybir.AluOpType
AX = mybir.AxisListType
NPBF = ml_dtypes.bfloat16

N, E, IN = 10000, 64000, 1247
H, D0, D1, OUT = 4, 256, 8, 6
HD0, HD1 = H * D0, H * D1          # 1024, 32
NC = 8
NPC = N // NC                       # 1250 nodes per core
WPC = (NPC + 127) // 128            # 10 windows per core
KA = IN + 1                         # 1248 augmented contraction dim
K0T = (KA + 127) // 128             # 10 k-tiles layer-0 dense
K1T = HD0 // 128                    # 8 k-tiles layer-1 dense
NPCP = WPC * 128                    # 1280 padded per-k block (DMA transposes)
WCATW = 3 * HD0 + HD1 + 8           # 3112 fused layer-0 output width
CROW = HD0 + 128                    # 1152 combined gather row (bf16)
# combined row layout: [0:1024]=fs, [1024:1028]=La, [1028:1032]=es,
# [1032:1064]=z, rest pad
AEPS = 1e-4

_compiled = {}
last_exec_ns = None
_last_in_maps = None
_last_res = None


def _wrows(w):
    return min(128, NPC - w * 128)


def _split_ranges(ph):
    """8 (c0, c1, col) ranges ordered (h,+),(h,-); greedy ACT/DVE split."""
    rng = []
    for h in range(H):
        rng.append((h * D0, h * D0 + ph[h], 2 * h))
        rng.append((h * D0 + ph[h], (h + 1) * D0, 2 * h + 1))
    # engine split: ACT ~5, DVE ~3 balancing lengths
    order = sorted(range(8), key=lambda i: -(rng[i][1] - rng[i][0]))
    act_t, dve_t = 570.0, 1100.0
    eng = [None] * 8
    for i in order:
        ln = rng[i][1] - rng[i][0]
        ca = (172 + ln) / 1.2
        cd = (120 + ln / 2) / 0.96
        if act_t + ca <= dve_t + cd:
            eng[i] = "act"; act_t += ca
        else:
            eng[i] = "dve"; dve_t += cd
    return [(c0, c1, col, eng[i]) for i, (c0, c1, col) in enumerate(rng)]


def _build_program(Ts, ph, comms=True):
    totT = sum(Ts)
    Tmax = max(Ts)
    toff = [sum(Ts[:w]) for w in range(WPC)]
    ranges = _split_ranges(ph)
    nc = bacc.Bacc("TRN2", target_bir_lowering=False, debug=False,
                   num_devices=NC)

    feat16 = nc.dram_tensor("feat16", [NPC, IN], BF16, kind="ExternalInput")
    wcat = nc.dram_tensor("wcat", [KA, WCATW], BF16, kind="ExternalInput")
    wk1c = nc.dram_tensor("wk1c", [HD0, 3 * HD1], BF16, kind="ExternalInput")
    wlin16 = nc.dram_tensor("wlin16", [2 * HD1, OUT], BF16,
                            kind="ExternalInput")
    a1bc = nc.dram_tensor("a1bc", [128, HD1], F32, kind="ExternalInput")
    a2sbc = nc.dram_tensor("a2sbc", [128, HD1], F32, kind="ExternalInput")
    a2dbc = nc.dram_tensor("a2dbc", [128, HD1], F32, kind="ExternalInput")
    b1bc = nc.dram_tensor("b1bc", [128, HD1], F32, kind="ExternalInput")
    blinbc = nc.dram_tensor("blinbc", [128, OUT], F32, kind="ExternalInput")
    seld = nc.dram_tensor("seld", [128, totT * 128], BF16,
                          kind="ExternalInput")
    selTd = nc.dram_tensor("selTd", [128, totT * 128], BF16,
                           kind="ExternalInput")
    srcidx = nc.dram_tensor("srcidx", [128, 8 * totT], I16,
                            kind="ExternalInput")
    out_ext = nc.dram_tensor("out", [NPC, OUT], F32, kind="ExternalOutput")
    import os
    DBG = os.environ.get("KDBG") == "1"
    if DBG:
        dbg_comb = nc.dram_tensor("dbg_comb", [NPC, CROW], BF16,
                                  kind="ExternalOutput")
        dbg_fd = nc.dram_tensor("dbg_fd", [NPC, HD0], BF16,
                                kind="ExternalOutput")
        dbg_led = nc.dram_tensor("dbg_led", [128, 8 * WPC], BF16,
                                 kind="ExternalOutput")
        dbg_fs1 = nc.dram_tensor("dbg_fs1", [NPC, 64], F32,
                                 kind="ExternalOutput")
        dbg_h1 = nc.dram_tensor("dbg_h1", [128, K1T * NPCP], BF16,
                                kind="ExternalOutput")
        dbg_e0 = nc.dram_tensor("dbg_e0", [128, 32], F32,
                                kind="ExternalOutput")
        dbg_ops = nc.dram_tensor("dbg_ops", [128, HD0 + 64], F32,
                                 kind="ExternalOutput")

    with tile.TileContext(nc) as tc:
        with tc.tile_pool(name="dram", bufs=1, space="DRAM") as dram, \
             tc.tile_pool(name="constp", bufs=1) as constp, \
             tc.tile_pool(name="work", bufs=2) as work:

            comb_c = dram.tile([NPC, CROW], BF16)
            fd_c = dram.tile([NPC, HD0], BF16)
            res_c = dram.tile([NPC, HD0], BF16)
            fs1p_c = dram.tile([NPC, 64], F32)
            ASP = "Shared" if comms else "Local"
            comb_full = dram.tile([N, CROW], BF16, addr_space=ASP)
            fs1p_full = dram.tile([N, 64], F32, addr_space=ASP)
            cs_bounce = dram.tile([1, IN], F32)
            cs_sum = dram.tile([1, IN], F32, addr_space=ASP)

            ident16 = constp.tile([128, 128], BF16)
            make_identity(nc, ident16[:])
            identf = constp.tile([128, 128], F32)
            make_identity(nc, identf[:])
            ones16 = constp.tile([128, 1], BF16)
            nc.vector.memset(ones16[:], 1.0)
            ones_row = constp.tile([1, 128], F32)
            nc.vector.memset(ones_row[:], 1.0)

            def load_const(name, dramt, shape, dt=F32):
                t = constp.tile(shape, dt, tag=name, name=name)
                nc.sync.dma_start(out=t[:], in_=dramt[:])
                return t
            a1b = load_const("a1b", a1bc, [128, HD1])
            a2sb = load_const("a2sb", a2sbc, [128, HD1])
            a2db = load_const("a2db", a2dbc, [128, HD1])
            b1b = load_const("b1b", b1bc, [128, HD1])
            blinb = load_const("blinb", blinbc, [128, OUT])
            wlsb = load_const("wlsb", wlin16, [2 * HD1, OUT], BF16)
            sidx = load_const("sidx", srcidx, [128, 8 * totT], I16)
            selb = load_const("selb", seld, [128, totT * 128], BF16)
            selTb = load_const("selTb", selTd, [128, totT * 128], BF16)

            ledw = constp.tile([128, 8 * WPC], BF16)   # [Lb(4)|ed(4)] per w
            catT = constp.tile([64, NPCP], BF16)
            fd1_t = [constp.tile([128, HD1], BF16, tag=f"fd1_{m}",
                                 name=f"fd1_{m}") for m in range(WPC)]
            res1_t = [constp.tile([128, HD1], F32, tag=f"res1_{m}",
                                  name=f"res1_{m}") for m in range(WPC)]

            # ================= P + D0 (hT / wcat alive) =================
            with tc.tile_pool(name="pd0", bufs=1) as pd0:
                hT = pd0.tile([128, K0T * NPCP], BF16)
                wks = [pd0.tile([128, WCATW], BF16, tag=f"wk{k}",
                                name=f"wk{k}") for k in range(K0T)]
                for k in range(K0T):
                    kw = min(128, KA - k * 128)
                    nc.sync.dma_start(out=wks[k][:kw, :],
                                      in_=wcat[k * 128:k * 128 + kw, :])

                with tc.tile_pool(name="pp", bufs=1) as pp, \
                     tc.tile_pool(name="psP", bufs=1, space="PSUM") as psP:
                    ftA = pp.tile([128, WPC * IN], BF16)
                    mskA = pp.tile([128, WPC * IN], BF16)
                    ncol = [(j * 512, min(512, IN - j * 512))
                            for j in range((IN + 511) // 512)]
                    cpss = [psP.tile([128, 512], F32, tag=f"cs{j}",
                                     name=f"cs{j}", space="PSUM")
                            for j in range(len(ncol))]
                    for m in range(WPC):
                        pr = _wrows(m)
                        nc.sync.dma_start(
                            out=ftA[:pr, m * IN:(m + 1) * IN],
                            in_=feat16[m * 128:m * 128 + pr, :])
                        for j, (c0, cw) in enumerate(ncol):
                            nc.tensor.matmul(
                                out=cpss[j][:1, :cw], lhsT=ones16[:pr, :],
                                rhs=ftA[:pr, m * IN + c0:m * IN + c0 + cw],
                                start=(m == 0), stop=(m == WPC - 1))
                    cs_sb = pp.tile([1, IN], F32, tag="cs_sb")
                    for j, (c0, cw) in enumerate(ncol):
                        nc.scalar.copy(out=cs_sb[:, c0:c0 + cw],
                                       in_=cpss[j][:1, :cw])
                    nc.gpsimd.dma_start(out=cs_bounce[:], in_=cs_sb[:])
                    if comms:
                        nc.gpsimd.collective_compute(
                            "AllReduce", OP.add,
                            replica_groups=[list(range(NC))],
                            ins=[cs_bounce[:]], outs=[cs_sum[:]])
                    else:
                        nc.gpsimd.dma_start(out=cs_sum[:], in_=cs_bounce[:])

                    # mean-independent: zero masks (hide AllReduce latency)
                    for m in range(WPC):
                        nc.vector.tensor_scalar(
                            out=mskA[:, m * IN:(m + 1) * IN],
                            in0=ftA[:, m * IN:(m + 1) * IN],
                            scalar1=0.0, scalar2=None, op0=OP.is_equal)

                    meanh = pp.tile([1, IN], F32, tag="meanh")
                    nc.sync.dma_start(out=meanh[:], in_=cs_sum[:])
                    nc.vector.tensor_scalar(out=meanh[:], in0=meanh[:],
                                            scalar1=0.5 / N, scalar2=None,
                                            op0=OP.mult)
                    meanb16 = pp.tile([128, IN], BF16, tag="meanb16")
                    for j, (c0, cw) in enumerate(ncol):
                        bps = psP.tile([128, 512], F32, tag="bps", name="bps",
                                       space="PSUM")
                        nc.tensor.matmul(out=bps[:, :cw], lhsT=ones_row[:, :],
                                         rhs=meanh[:, c0:c0 + cw],
                                         start=True, stop=True)
                        (nc.scalar.copy if j % 2 else nc.vector.tensor_copy)(
                            out=meanb16[:, c0:c0 + cw], in_=bps[:, :cw])

                    junk16 = pp.tile([128, IN], BF16, tag="junk16")
                    for m in range(WPC):
                        pr = _wrows(m)
                        msk = mskA[:, m * IN:(m + 1) * IN]
                        nc.vector.tensor_tensor(out=msk, in0=msk,
                                                in1=meanb16[:], op=OP.mult)
                        h16 = pp.tile([128, K0T * 128], BF16, tag="h16",
                                      name="h16", bufs=2)
                        nc.vector.memset(h16[:, IN:KA], 1.0)
                        nc.vector.tensor_tensor(
                            out=h16[:, 0:IN], in0=ftA[:, m * IN:(m + 1) * IN],
                            in1=msk, op=OP.add)
                        rs = work.tile([128, 1], F32, tag="rs")
                        nc.scalar.activation(out=junk16[:pr, :],
                                             in_=h16[:pr, 0:IN], func=AF.Abs,
                                             accum_out=rs[:pr, :])
                        nc.vector.tensor_scalar(out=rs[:pr, :], in0=rs[:pr, :],
                                                scalar1=1e-12, scalar2=None,
                                                op0=OP.max)
                        rinv = work.tile([128, 1], F32, tag="rinv")
                        nc.vector.reciprocal(out=rinv[:pr, :], in_=rs[:pr, :])
                        nc.vector.tensor_scalar(out=h16[:pr, 0:IN],
                                                in0=h16[:pr, 0:IN],
                                                scalar1=rinv[:pr, 0:1],
                                                scalar2=None, op0=OP.mult)
                        for k in range(K0T):
                            kw = min(128, KA - k * 128)
                            eng = (nc.sync, nc.scalar)[k % 2]
                            eng.dma_start_transpose(
                                out=hT[:, k * NPCP + m * 128:
                                       k * NPCP + m * 128 + 128],
                                in_=h16[:, k * 128:(k + 1) * 128])

                # ---------- D0 fused dense ----------
                with tc.tile_pool(name="d0w", bufs=2) as d0w, \
                     tc.tile_pool(name="psD", bufs=1, space="PSUM") as psD:
                    for m in range(WPC):
                        pr = _wrows(m)
                        ps6 = [psD.tile([128, 512], F32, tag=f"d0p{j}",
                                        name=f"d0p{j}", space="PSUM")
                               for j in range(6)]
                        p40 = psD.tile([128, 512], F32, tag="d0s", name="d0s",
                                       space="PSUM")
                        for k in range(K0T):
                            kw = min(128, KA - k * 128)
                            lhs = hT[:kw, k * NPCP + m * 128:
                                     k * NPCP + m * 128 + pr]
                            for j in range(6):
                                nc.tensor.matmul(
                                    out=ps6[j][:pr, :], lhsT=lhs,
                                    rhs=wks[k][:kw, j * 512:(j + 1) * 512],
                                    start=(k == 0), stop=(k == K0T - 1))
                            nc.tensor.matmul(
                                out=p40[:pr, 0:40], lhsT=lhs,
                                rhs=wks[k][:kw, 3072:3112],
                                start=(k == 0), stop=(k == K0T - 1))
                        combsb = d0w.tile([128, CROW], BF16, tag="combsb",
                                          name="combsb")
                        fdsb = d0w.tile([128, HD0], BF16, tag="fdsb",
                                        name="fdsb")
                        ressb = d0w.tile([128, HD0], BF16, tag="ressb",
                                         name="ressb")
                        nc.scalar.copy(out=combsb[:pr, 0:512],
                                       in_=ps6[0][:pr, :])
                        nc.vector.tensor_copy(out=combsb[:pr, 512:1024],
                                              in_=ps6[1][:pr, :])
                        nc.vector.tensor_copy(out=fdsb[:pr, 0:512],
                                              in_=ps6[2][:pr, :])
                        nc.scalar.copy(out=fdsb[:pr, 512:1024],
                                       in_=ps6[3][:pr, :])
                        nc.vector.tensor_copy(out=ressb[:pr, 0:512],
                                              in_=ps6[4][:pr, :])
                        nc.scalar.copy(out=ressb[:pr, 512:1024],
                                       in_=ps6[5][:pr, :])
                        # z vector pieces: es/ed from z (p40[:,0:32])
                        nc.scalar.copy(out=combsb[:pr, 1032:1064],
                                       in_=p40[:pr, 0:32])
                        tmp = work.tile([128, HD1], F32, tag="ztmp",
                                        name="ztmp")
                        nc.vector.tensor_tensor(out=tmp[:pr, :],
                                                in0=p40[:pr, 0:32],
                                                in1=a2sb[:pr, :], op=OP.mult)
                        esf = work.tile([128, H], F32, tag="esf", name="esf")
                        nc.vector.tensor_reduce(
                            out=esf[:pr, :],
                            in_=tmp[:pr, :].rearrange("p (h d) -> p h d", h=H),
                            axis=AX.X, op=OP.add)
                        nc.vector.tensor_copy(out=combsb[:pr, 1028:1032],
                                              in_=esf[:pr, :])
                        nc.vector.tensor_tensor(out=tmp[:pr, :],
                                                in0=p40[:pr, 0:32],
                                                in1=a2db[:pr, :], op=OP.mult)
                        edf = work.tile([128, H], F32, tag="edf", name="edf")
                        nc.vector.tensor_reduce(
                            out=edf[:pr, :],
                            in_=tmp[:pr, :].rearrange("p (h d) -> p h d", h=H),
                            axis=AX.X, op=OP.add)
                        nc.vector.tensor_copy(
                            out=ledw[:pr, 8 * m + 4:8 * m + 8],
                            in_=edf[:pr, :])
                        nc.scalar.copy(out=combsb[:pr, 1024:1028],
                                       in_=p40[:pr, 32:36])
                        nc.scalar.copy(out=ledw[:pr, 8 * m:8 * m + 4],
                                       in_=p40[:pr, 36:40])
                        nc.sync.dma_start(
                            out=comb_c[m * 128:m * 128 + pr, :],
                            in_=combsb[:pr, :])
                        nc.gpsimd.dma_start(
                            out=fd_c[m * 128:m * 128 + pr, :],
                            in_=fdsb[:pr, :])
                        nc.scalar.dma_start(
                            out=res_c[m * 128:m * 128 + pr, :],
                            in_=ressb[:pr, :])
                if DBG:
                    nc.sync.dma_start(out=dbg_comb[:], in_=comb_c[:])
                    nc.sync.dma_start(out=dbg_fd[:], in_=fd_c[:])
                if comms:
                    nc.gpsimd.collective_compute(
                        "AllGather", OP.bypass,
                        replica_groups=[list(range(NC))],
                        ins=[comb_c[:]], outs=[comb_full[:]])
                else:
                    for r in range(NC):
                        nc.sync.dma_start(
                            out=comb_full[r * NPC:(r + 1) * NPC, :],
                            in_=comb_c[:, :])

            # ================= E0 + D1 (h1T alive) =================
            with tc.tile_pool(name="e0pool", bufs=1) as e0p:
                h1T = e0p.tile([128, K1T * NPCP], BF16)
                with tc.tile_pool(name="psE0", bufs=1, space="PSUM") as psE0:
                    for w in range(WPC):
                        T = Ts[w]
                        nloc = _wrows(w)
                        co = toff[w]
                        fsg = e0p.tile([128, Tmax * CROW], BF16, tag="fsg",
                                       name="fsg", bufs=2)
                        nc.gpsimd.dma_gather(
                            out_ap=fsg[:].rearrange("p (t e) -> p t e",
                                                    t=Tmax)[:, :T, :],
                            in_ap=comb_full[:],
                            idxs_ap=sidx[:, 8 * co:8 * (co + T)],
                            num_idxs=T * 128, num_idxs_reg=T * 128,
                            elem_size=CROW)
                        fdw = e0p.tile([128, HD0], BF16, tag="fdw",
                                       name="fdw", bufs=2)
                        nc.sync.dma_start(out=fdw[:nloc, :],
                                          in_=fd_c[w * 128:w * 128 + nloc, :])
                        resw = e0p.tile([128, HD0], BF16, tag="resw",
                                        name="resw", bufs=2)
                        nc.scalar.dma_start(
                            out=resw[:nloc, :],
                            in_=res_c[w * 128:w * 128 + nloc, :])

                        o_ps = psE0.tile([128, HD0], F32, tag="o_ps",
                                         name="o_ps", space="PSUM")
                        smA = psE0.tile([128, 512], F32, tag="smA",
                                        name="smA", space="PSUM")
                        # smA: [0:4]=dn [4:8]=dn2 [8:40]=oz (single group)
                        for t in range(T):
                            sel = selb[:, (co + t) * 128:(co + t + 1) * 128]
                            selT = selTb[:, (co + t) * 128:(co + t + 1) * 128]
                            ft = fsg[:, (t * CROW):(t * CROW + CROW)]
                            shat = psE0.tile([128, HD0], F32, tag="shat",
                                             name="shat", space="PSUM",
                                             bufs=2)
                            for j in range(2):
                                nc.tensor.matmul(
                                    out=shat[:, j * 512:(j + 1) * 512],
                                    lhsT=selT[:nloc, :],
                                    rhs=fdw[:nloc, j * 512:(j + 1) * 512],
                                    start=True, stop=False)
                            smB = psE0.tile([128, 512], F32, tag="smB",
                                            name="smB", space="PSUM")
                            nc.tensor.matmul(
                                out=smB[:, 0:8], lhsT=selT[:nloc, :],
                                rhs=ledw[:nloc, 8 * w:8 * w + 8],
                                start=True, stop=True)
                            for j in range(2):
                                nc.tensor.matmul(
                                    out=shat[:, j * 512:(j + 1) * 512],
                                    lhsT=ident16[:, :],
                                    rhs=ft[:, j * 512:(j + 1) * 512],
                                    start=False, stop=True)
                            acc = work.tile([128, 8], F32, tag="acc",
                                            name="acc")
                            ajunk = work.tile([128, D0], BF16, tag="ajunk",
                                              name="ajunk")
                            djunk = work.tile([128, D0], BF16, tag="djunk",
                                              name="djunk")
                            for (c0, c1, col, eng) in ranges:
                                if c1 <= c0:
                                    nc.vector.memset(acc[:, col:col + 1], 0.0)
                                    continue
                                if eng == "act":
                                    nc.scalar.activation(
                                        out=ajunk[:, 0:c1 - c0],
                                        in_=shat[:, c0:c1], func=AF.Relu,
                                        accum_out=acc[:, col:col + 1])
                                else:
                                    nc.vector.tensor_scalar(
                                        out=djunk[:, 0:c1 - c0],
                                        in0=shat[:, c0:c1], scalar1=0.0,
                                        scalar2=0.0, op0=OP.max,
                                        op1=OP.add,
                                        accum_out=acc[:, col:col + 1])
                            u = work.tile([128, H], F32, tag="u", name="u")
                            nc.vector.tensor_tensor(
                                out=u[:], in0=ft[:, 1024:1028],
                                in1=smB[:, 0:4], op=OP.add)
                            t3 = work.tile([128, H], F32, tag="t3", name="t3")
                            nc.vector.tensor_tensor(
                                out=t3[:],
                                in0=acc[:].rearrange("p (h s) -> p h s",
                                                     h=H)[:, :, 0],
                                in1=acc[:].rearrange("p (h s) -> p h s",
                                                     h=H)[:, :, 1],
                                op=OP.subtract)
                            nc.vector.tensor_tensor(out=t3[:], in0=t3[:],
                                                    in1=u[:], op=OP.add)
                            el = work.tile([128, H], F32, tag="el",
                                           name="el")
                            nc.scalar.activation(out=el[:], in_=t3[:],
                                                 func=AF.Exp, scale=0.8)
                            el8 = work.tile([128, 64], BF16, tag="el8",
                                            name="el8")
                            nc.vector.tensor_copy(out=el8[:, 0:4], in_=el[:])
                            if DBG and w == 0 and t == 0:
                                d0t = e0p.tile([128, 32], F32, tag="d0t")
                                nc.vector.tensor_copy(out=d0t[:, 0:8],
                                                      in_=acc[:])
                                nc.vector.tensor_copy(out=d0t[:, 8:12],
                                                      in_=u[:])
                                nc.vector.tensor_copy(out=d0t[:, 12:16],
                                                      in_=t3[:])
                                nc.vector.tensor_copy(out=d0t[:, 16:20],
                                                      in_=el[:])
                                nc.vector.tensor_copy(out=d0t[:, 20:24],
                                                      in_=shat[:, 0:4])
                                nc.vector.tensor_copy(out=d0t[:, 28:32],
                                                      in_=el8[:, 0:4])
                                nc.sync.dma_start(out=dbg_e0[:], in_=d0t[:])
                            # z stream
                            lg2 = work.tile([128, H], F32, tag="lg2",
                                            name="lg2")
                            nc.vector.tensor_tensor(
                                out=lg2[:], in0=ft[:, 1028:1032],
                                in1=smB[:, 4:8], op=OP.add)
                            nc.scalar.activation(out=lg2[:], in_=lg2[:],
                                                 func=AF.Prelu, alpha=0.2)
                            el2 = work.tile([128, H], F32, tag="el2",
                                            name="el2")
                            nc.scalar.activation(out=el2[:], in_=lg2[:],
                                                 func=AF.Exp)
                            nc.vector.tensor_copy(out=el8[:, 4:8],
                                                  in_=el2[:])

                            nc.vector.tensor_tensor(
                                out=el8[:, 8:40].rearrange(
                                    "p (h d) -> p h d", h=H),
                                in0=ft[:, 1032:1064].rearrange(
                                    "p (h d) -> p h d", h=H),
                                in1=el2[:].to_broadcast([128, H, D1]),
                                op=OP.mult)
                            nc.tensor.matmul(out=smA[:, 0:40],
                                             lhsT=sel[:, :],
                                             rhs=el8[:, 0:40],
                                             start=(t == 0),
                                             stop=(t == T - 1))
                            for h in range(H):
                                nc.vector.tensor_scalar(
                                    out=ft[:, h * D0:(h + 1) * D0],
                                    in0=ft[:, h * D0:(h + 1) * D0],
                                    scalar1=el[:, h:h + 1], scalar2=None,
                                    op0=OP.mult)
                            for j in range(2):
                                nc.tensor.matmul(
                                    out=o_ps[:, j * 512:(j + 1) * 512],
                                    lhsT=sel[:, :],
                                    rhs=ft[:, j * 512:(j + 1) * 512],
                                    start=(t == 0), stop=(t == T - 1))

                        if DBG and w == 0:
                            dops = e0p.tile([128, HD0 + 64], F32,
                                            tag="dops")
                            nc.vector.tensor_copy(out=dops[:, 0:HD0],
                                                  in_=o_ps[:])
                            nc.vector.tensor_copy(out=dops[:, HD0:HD0 + 40],
                                                  in_=smA[:, 0:40])
                            nc.sync.dma_start(out=dbg_ops[:], in_=dops[:])
                        idn = work.tile([128, H], F32, tag="idn")
                        nc.vector.tensor_scalar(out=idn[:], in0=smA[:, 0:4],
                                                scalar1=1e-9, scalar2=None,
                                                op0=OP.max)
                        nc.vector.reciprocal(out=idn[:], in_=idn[:])
                        idn2 = work.tile([128, H], F32, tag="idn2")
                        nc.vector.tensor_scalar(out=idn2[:], in0=smA[:, 4:8],
                                                scalar1=1e-9, scalar2=None,
                                                op0=OP.max)
                        nc.vector.reciprocal(out=idn2[:], in_=idn2[:])
                        ho = work.tile([128, HD0], BF16, tag="ho")
                        for h in range(H):
                            nc.vector.tensor_scalar(
                                out=ho[:nloc, h * D0:(h + 1) * D0],
                                in0=o_ps[:nloc, h * D0:(h + 1) * D0],
                                scalar1=idn[:nloc, h:h + 1], scalar2=None,
                                op0=OP.mult)
                        nc.vector.tensor_tensor(out=ho[:nloc, :],
                                                in0=ho[:nloc, :],
                                                in1=resw[:nloc, :], op=OP.add)
                        nc.scalar.activation(out=ho[:nloc, :],
                                             in_=ho[:nloc, :], func=AF.Relu)
                        for k in range(K1T):
                            eng = (nc.sync, nc.scalar)[k % 2]
                            eng.dma_start_transpose(
                                out=h1T[:, k * NPCP + w * 128:
                                        k * NPCP + w * 128 + 128],
                                in_=ho[:, k * 128:(k + 1) * 128])
                        ozs = work.tile([128, HD1], F32, tag="ozs")
                        nc.vector.tensor_tensor(
                            out=ozs[:nloc, :].rearrange("p (h d) -> p h d",
                                                        h=H),
                            in0=smA[:nloc, 8:40].rearrange("p (h d) -> p h d",
                                                           h=H),
                            in1=idn2[:nloc, :].to_broadcast([nloc, H, D1]),
                            op=OP.mult)
                        zt = psE0.tile([128, 512], F32, tag="smB", name="zt",
                                       space="PSUM")
                        nc.tensor.transpose(out=zt[:HD1, 0:nloc],
                                            in_=ozs[:nloc, :],
                                            identity=identf[:nloc, :nloc])
                        nc.scalar.copy(out=catT[0:HD1, w * 128:w * 128 + nloc],
                                       in_=zt[:HD1, :nloc])

                # ---------- D1 fused dense ----------
                with tc.tile_pool(name="psD1", bufs=2, space="PSUM") as psD1:
                    wk1 = constp.tile([128, K1T * 3 * HD1], BF16, tag="wk1")
                    for k in range(K1T):
                        nc.sync.dma_start(
                            out=wk1[:, k * 96:(k + 1) * 96],
                            in_=wk1c[k * 128:(k + 1) * 128, :])
                    for m in range(WPC):
                        pr = _wrows(m)
                        p1 = psD1.tile([128, 512], F32, tag="d1p", name="d1p",
                                       space="PSUM")
                        for k in range(K1T):
                            nc.tensor.matmul(
                                out=p1[:pr, 0:96],
                                lhsT=h1T[:, k * NPCP + m * 128:
                                         k * NPCP + m * 128 + pr],
                                rhs=wk1[:, k * 96:(k + 1) * 96],
                                start=(k == 0), stop=(k == K1T - 1))
                        f1 = work.tile([128, 64], F32, tag="f1", name="f1")
                        nc.vector.memset(f1[:], 0.0)
                        nc.scalar.copy(out=f1[:pr, 0:HD1], in_=p1[:pr, 0:32])
                        nc.sync.dma_start(out=fs1p_c[m * 128:m * 128 + pr, :],
                                          in_=f1[:pr, :])
                        nc.vector.tensor_copy(out=fd1_t[m][:pr, :],
                                              in_=p1[:pr, 32:64])
                        nc.vector.tensor_tensor(out=res1_t[m][:pr, :],
                                                in0=p1[:pr, 64:96],
                                                in1=b1b[:pr, :], op=OP.add)
                if DBG:
                    nc.sync.dma_start(out=dbg_fs1[:], in_=fs1p_c[:])
                    nc.sync.dma_start(out=dbg_led[:], in_=ledw[:])
                    nc.sync.dma_start(out=dbg_h1[:], in_=h1T[:])
                if comms:
                    nc.gpsimd.collective_compute(
                        "AllGather", OP.bypass,
                        replica_groups=[list(range(NC))],
                        ins=[fs1p_c[:]], outs=[fs1p_full[:]])
                else:
                    for r in range(NC):
                        nc.sync.dma_start(
                            out=fs1p_full[r * NPC:(r + 1) * NPC, :],
                            in_=fs1p_c[:, :])

            # ================= E1 =================
            with tc.tile_pool(name="e1pool", bufs=1) as e1p, \
                 tc.tile_pool(name="psE1", bufs=1, space="PSUM") as psE1:
                for w in range(WPC):
                    T = Ts[w]
                    nloc = _wrows(w)
                    co = toff[w]
                    f1g = e1p.tile([128, Tmax * 64], F32, tag="f1g",
                                   name="f1g", bufs=2)
                    nc.gpsimd.dma_gather(
                        out_ap=f1g[:].rearrange("p (t e) -> p t e",
                                                t=Tmax)[:, :T, :],
                        in_ap=fs1p_full[:],
                        idxs_ap=sidx[:, 8 * co:8 * (co + T)],
                        num_idxs=T * 128, num_idxs_reg=T * 128, elem_size=64)
                    o1a = psE1.tile([128, 512], F32, tag="o1a", name="o1a",
                                    space="PSUM")
                    # o1a: [0:4]=dn3 [4:8]=pad [8:40]=o1 (single group)
                    for t in range(T):
                        sel = selb[:, (co + t) * 128:(co + t + 1) * 128]
                        selT = selTb[:, (co + t) * 128:(co + t + 1) * 128]
                        fdx = psE1.tile([128, 512], F32, tag="fdx1",
                                        name="fdx1", space="PSUM", bufs=2)
                        nc.tensor.matmul(out=fdx[:, 0:HD1],
                                         lhsT=selT[:nloc, :],
                                         rhs=fd1_t[w][:nloc, :], start=True,
                                         stop=True)
                        tt = work.tile([128, HD1], F32, tag="tt1")
                        nc.vector.tensor_tensor(
                            out=tt[:], in0=f1g[:, t * 64:t * 64 + HD1],
                            in1=fdx[:, 0:HD1], op=OP.add)
                        nc.scalar.activation(out=tt[:], in_=tt[:],
                                             func=AF.Prelu, alpha=0.2)
                        nc.vector.tensor_tensor(out=tt[:], in0=tt[:],
                                                in1=a1b[:], op=OP.mult)
                        lg = work.tile([128, H], F32, tag="lg3")
                        nc.vector.tensor_reduce(
                            out=lg[:],
                            in_=tt[:].rearrange("p (h d) -> p h d", h=H),
                            axis=AX.X, op=OP.add)
                        el = work.tile([128, H], F32, tag="el3", name="el3")
                        nc.scalar.activation(out=el[:], in_=lg[:],
                                             func=AF.Exp)
                        el16 = work.tile([128, 64], BF16, tag="el316",
                                         name="el316")
                        nc.vector.tensor_copy(out=el16[:, 0:4], in_=el[:])
                        nc.vector.memset(el16[:, 4:8], 0.0)
                        nc.vector.tensor_tensor(
                            out=el16[:, 8:40].rearrange("p (h d) -> p h d",
                                                        h=H),
                            in0=f1g[:, t * 64:t * 64 + HD1].rearrange(
                                "p (h d) -> p h d", h=H),
                            in1=el[:].to_broadcast([128, H, D1]),
                            op=OP.mult)
                        nc.tensor.matmul(out=o1a[:, 0:40], lhsT=sel[:, :],
                                         rhs=el16[:, 0:40], start=(t == 0),
                                         stop=(t == T - 1))
                    idn = work.tile([128, H], F32, tag="idn3")
                    nc.vector.tensor_scalar(out=idn[:], in0=o1a[:, 0:4],
                                            scalar1=1e-9, scalar2=None,
                                            op0=OP.max)
                    nc.vector.reciprocal(out=idn[:], in_=idn[:])
                    oo = work.tile([128, HD1], F32, tag="oo")
                    nc.vector.tensor_tensor(
                        out=oo[:nloc, :].rearrange("p (h d) -> p h d", h=H),
                        in0=o1a[:nloc, 8:40].rearrange("p (h d) -> p h d",
                                                       h=H),
                        in1=idn[:nloc, :].to_broadcast([nloc, H, D1]),
                        op=OP.mult)
                    nc.vector.tensor_tensor(out=oo[:nloc, :],
                                            in0=oo[:nloc, :],
                                            in1=res1_t[w][:nloc, :],
                                            op=OP.add)
                    nc.scalar.activation(out=oo[:nloc, :],
                                         in_=oo[:nloc, :], func=AF.Relu)
                    tp1 = psE1.tile([128, 512], F32, tag="tp1", name="tp1",
                                    space="PSUM")
                    nc.tensor.transpose(out=tp1[:HD1, :nloc],
                                        in_=oo[:nloc, :],
                                        identity=identf[:nloc, :nloc])
                    nc.scalar.copy(out=catT[HD1:2 * HD1,
                                            w * 128:w * 128 + nloc],
                                   in_=tp1[:HD1, :nloc])

            # ================= F =================
            with tc.tile_pool(name="psF", bufs=2, space="PSUM") as psF:
                for m in range(WPC):
                    pr = _wrows(m)
                    fp = psF.tile([128, 512], F32, tag="fin", name="fin",
                                  space="PSUM")
                    nc.tensor.matmul(out=fp[:pr, 0:OUT],
                                     lhsT=catT[:, m * 128:m * 128 + pr],
                                     rhs=wlsb[:], start=True, stop=True)
                    osb = work.tile([128, OUT], F32, tag="osb")
                    nc.vector.tensor_tensor(out=osb[:pr, :],
                                            in0=fp[:pr, 0:OUT],
                                            in1=blinb[:pr, :], op=OP.add)
                    nc.sync.dma_start(out=out_ext[m * 128:m * 128 + pr, :],
                                      in_=osb[:pr, :])

    nc.compile()
    return nc


def _prep_edges(src, dst):
    order = np.argsort(dst, kind="stable")
    ss = src[order].astype(np.int64)
    ds = dst[order].astype(np.int64)
    cnt = np.zeros((NC, WPC), np.int64)
    bounds = {}
    for c in range(NC):
        for w in range(WPC):
            lo = c * NPC + w * 128
            hi = min(c * NPC + (w + 1) * 128, (c + 1) * NPC)
            e0 = np.searchsorted(ds, lo, side="left")
            e1 = np.searchsorted(ds, hi, side="left")
            cnt[c, w] = e1 - e0
            bounds[(c, w)] = (e0, e1)
    nws = [int(cnt[:, w].max()) for w in range(WPC)]
    Ts = [max(1, math.ceil(nv / 128)) for nv in nws]
    totT = sum(Ts)
    eyeext = np.zeros((129, 128), np.float32)
    eyeext[1:] = np.eye(128, dtype=np.float32)
    per_core = []
    for c in range(NC):
        sidx = np.zeros((128, 8 * totT), np.int16)
        selm = np.zeros((128, totT * 128), NPBF)
        selTm = np.zeros((128, totT * 128), NPBF)
        co = 0
        for w in range(WPC):
            T = Ts[w]
            e0, e1 = bounds[(c, w)]
            k = e1 - e0
            slots = T * 128
            s = np.zeros(slots, np.int16)
            d = np.full(slots, -1, np.int64)
            s[:k] = ss[e0:e1]
            d[:k] = ds[e0:e1] - (c * NPC + w * 128)
            cols = s.reshape(8 * T, 16).T
            sidx[:, 8 * co:8 * (co + T)] = np.tile(cols, (8, 1))
            dm = d.reshape(T, 128)
            for t in range(T):
                blk = eyeext[dm[t] + 1]           # [128e, 128v]
                selm[:, (co + t) * 128:(co + t + 1) * 128] = blk.astype(NPBF)
                selTm[:, (co + t) * 128:(co + t + 1) * 128] = \
                    blk.T.astype(NPBF)
            co += T
        per_core.append((sidx, selm, selTm))
    return Ts, per_core


def kernel(features, src, dst, textMask, audioMask, videoMask, W2, a2,
           Wl0, Wr0, a0, Wres0, b0, Wl1, Wr1, a1, Wres1, b1, Wlin, blin):
    features = np.asarray(features, np.float32)
    src = np.asarray(src, np.int32)
    dst = np.asarray(dst, np.int32)
    a0f = np.asarray(a0, np.float32)

    # a0 sign-split: |a0| folded into layer-0 weights, columns permuted
    # positives-first per head, compensated in layer-1 rows
    absa = np.maximum(np.abs(a0f), AEPS)
    sgn = np.where(a0f >= 0, 1.0, -1.0).astype(np.float32)
    perm = np.zeros((H, D0), np.int64)
    ph = []
    for h in range(H):
        pos = np.where(sgn[h] > 0)[0]
        neg = np.where(sgn[h] <= 0)[0]
        perm[h] = np.concatenate([pos, neg])
        ph.append(len(pos))
    permflat = np.concatenate([perm[h] + h * D0 for h in range(H)])
    absflat = np.concatenate([absa[h, perm[h]] for h in range(H)])

    Ts, per_core = _prep_edges(src, dst)
    import os as _os
    key = (tuple(Ts), tuple(ph), _os.environ.get("KDBG"))
    if key not in _compiled:
        _compiled.clear()
        _compiled[key] = _build_program(Ts, ph)
    nc = _compiled[key]

    maskSum = (np.asarray(textMask) + np.asarray(audioMask)
               + np.asarray(videoMask)).astype(np.float32)

    def aug(Wm, brow=None):
        o = np.zeros((KA, Wm.shape[1]), np.float32)
        o[:IN] = Wm * maskSum[:, None]
        if brow is not None:
            o[IN] = brow
        return o

    wl0m = np.asarray(Wl0, np.float32)
    wr0m = np.asarray(Wr0, np.float32)
    wres0m = np.asarray(Wres0, np.float32)
    b0f = np.asarray(b0, np.float32)
    # block-diag a0 for La/Lb linear columns (x0.25 folded)
    a0bd = np.zeros((HD0, H), np.float32)
    for h in range(H):
        a0bd[h * D0:(h + 1) * D0, h] = a0f[h]
    w2flat = np.asarray(W2, np.float32).transpose(1, 0, 2).reshape(IN, HD1)
    wcat = np.zeros((KA, WCATW), np.float32)
    wcat[:, 0:HD0] = aug(wl0m[:, permflat] * absflat[None, :])
    wcat[:, HD0:2 * HD0] = aug(wr0m[:, permflat] * absflat[None, :])
    wcat[:, 2 * HD0:3 * HD0] = aug(wres0m[:, permflat] * absflat[None, :],
                                   b0f[permflat] * absflat)
    wcat[:, 3072:3104] = aug(w2flat)
    wcat[:, 3104:3108] = aug(wl0m @ a0bd) * 0.25
    wcat[:, 3108:3112] = aug(wr0m @ a0bd) * 0.25

    # layer-1 compensation: rows permuted and divided by |a0|
    def comp1(Wm):
        Wm = np.asarray(Wm, np.float32)
        return Wm[permflat, :] / absflat[:, None]
    wk1c = np.concatenate([comp1(Wl1), comp1(Wr1), comp1(Wres1)],
                          axis=1).astype(NPBF)

    shared = {
        "wcat": wcat.astype(NPBF),
        "wk1c": wk1c,
        "wlin16": np.asarray(Wlin, np.float32).astype(NPBF),
        "a1bc": np.tile(np.asarray(a1, np.float32).reshape(1, HD1), (128, 1)),
        "a2sbc": np.tile(np.asarray(a2, np.float32)[:, :D1].reshape(1, HD1),
                         (128, 1)),
        "a2dbc": np.tile(np.asarray(a2, np.float32)[:, D1:].reshape(1, HD1),
                         (128, 1)),
        "b1bc": np.tile(np.asarray(b1, np.float32).reshape(1, HD1), (128, 1)),
        "blinbc": np.tile(np.asarray(blin, np.float32).reshape(1, OUT),
                          (128, 1)),
    }
    in_maps = []
    for c in range(NC):
        sidx, selm, selTm = per_core[c]
        m = dict(shared)
        m["feat16"] = np.ascontiguousarray(
            features[c * NPC:(c + 1) * NPC]).astype(NPBF)
        m["srcidx"] = sidx
        m["seld"] = selm
        m["selTd"] = selTm
        in_maps.append(m)

    global _last_in_maps, _last_res
    _last_in_maps = in_maps
    res = run_bass_kernel_spmd(nc, in_maps, list(range(NC)))
    _last_res = res
    global last_exec_ns
    last_exec_ns = getattr(res, "exec_time_ns", None)
    return np.concatenate(
        [np.asarray(res.results[c]["out"]) for c in range(NC)], axis=0)
